# revision 3
# baseline (speedup 1.0000x reference)
"""Causal single-head attention (B=4, S=2048, D=1024) on 8 trn2 NeuronCores.

Sharding: core = (batch b, parity h).  Each core owns the 1024 queries of
batch b in 256-row blocks {2t+h : t=0..3} (interleaved for causal load
balance).  Projections are split across the pair: each core projects Q for
its own rows, K for its own half of the sequence (h=0: keys 0..1023, h=1:
keys 1024..2047), and V for its own rows; K and V halves are pair-wise
all-gathered, both hidden under the remaining projection work.

All inputs are pre-cast to bf16 on the host (the on-chip matmuls are bf16
anyway), halving input HBM traffic and removing the f32 staging casts.

On-chip dataflow (per core, SPMD-uniform):
  proj:  V_own(part)   (first quarter, starts as soon as 2MB of loads land)
         KT_own[e,k]  = Wk^T xk^T    -> AllGather #1 -> KT4[r][e,k]
         V_own rest:  V_own[k,e] = xq^T Wv -> AllGather #2 -> V4[r][j]
         QT[e,q]      = Wq^T xq^T
  attention per slot pair p (queries 512p..512p+511, nsh = 4(2p+1)):
         scoresT[k,q] = KT^T QT  over k-tiles 0..nsh-1 at N=512
                        (+ 4 extra k-tiles at N=256 for the later slot)
         expT = exp(scoresT/32) * mask    (multiplicative 0/1 causal mask)
         den[1,q] += ones^T expT          (matmul; [q]-major via DRAM
                                           roundtrip -> rinv[q,1])
         out[q,e]  = sum_k expT[k,q]^T V[k,e]   <- PV with expT stationary,
                     accumulated per 128-query tile directly in [q,e]
                     orientation (no PE transposes), scaled by rinv on DVE
                     and stored bf16.
"""

import os
import sys
from contextlib import ExitStack

import numpy as np
import ml_dtypes

import concourse.bass as bass
import concourse.mybir as mybir
import concourse.tile as tile
from concourse import bacc
from concourse import bass_utils

B, S, D = 4, 2048, 1024
P = 128
QB = 256          # queries per slot
NSLOT = 4         # slots per core
NQ = QB * NSLOT   # queries per core
NCORES = 8
F32 = mybir.dt.float32
BF16 = mybir.dt.bfloat16
SCALE = 1.0 / 32.0  # 1/sqrt(D)


def _build_kernel():
    nc = bacc.Bacc("TRN2", target_bir_lowering=False, debug=False,
                   num_devices=NCORES)

    xkd = nc.dram_tensor("xkd", [2, P, 8, 512], BF16, kind="ExternalInput").ap()
    xqd = nc.dram_tensor("xqd", [2, P, 8, 512], BF16, kind="ExternalInput").ap()
    wqd = nc.dram_tensor("wqd", [P, 8, D], BF16, kind="ExternalInput").ap()
    wkd = nc.dram_tensor("wkd", [P, 8, D], BF16, kind="ExternalInput").ap()
    wvd = nc.dram_tensor("wvd", [2, P, 8, 512], BF16, kind="ExternalInput").ap()
    maskT = nc.dram_tensor("maskT", [P, 4, 512], BF16, kind="ExternalInput").ap()
    out = nc.dram_tensor("out", [NQ, D], BF16, kind="ExternalOutput").ap()
    # cores 2b (h=0) and 2b+1 (h=1) of batch b exchange K and V halves
    GROUPS = [[0, 1], [2, 3], [4, 5], [6, 7]]

    with tile.TileContext(nc) as tc, ExitStack() as ctx:
        const = ctx.enter_context(tc.tile_pool(name="const", bufs=1))
        persist = ctx.enter_context(tc.tile_pool(name="persist", bufs=1))

        ones = const.tile([P, 1], BF16)
        nc.gpsimd.memset(ones[:], 1.0)
        mask_sb = const.tile([P, 4, 512], BF16)

        QT = persist.tile([P, 8, NQ], BF16)      # [e_in_tile, e_tile, q]
        KT4 = persist.tile([P, 2, 8, 1024], BF16)  # [e_in_tile, parity, e_tile, k]
        V4 = persist.tile([P, 2, 8, D], BF16)    # [k_in_tile, parity, j, e]
        denT = persist.tile([P, 2 * NSLOT], F32)
        rinv = persist.tile([P, 2 * NSLOT], F32)

        # ---------------- projection phase ----------------
        # Order: a quarter of V_own (smallest input footprint -> earliest
        # PE start), K_own (so AllGather #1 launches early; scores need the
        # gathered KT first), rest of V_own (AllGather #2), then Q.  Both
        # collectives hide under the remaining projections.
        with tc.tile_pool(name="wsb", bufs=1) as wsb_pool, \
             tc.tile_pool(name="xsb", bufs=1) as xsb_pool, \
             tc.tile_pool(name="vhp", bufs=1) as vh_pool, \
             tc.tile_pool(name="ccdram", bufs=1, space="DRAM") as ccdram, \
             tc.tile_pool(name="pproj", bufs=4, space="PSUM") as pproj:

            wq_sb = wsb_pool.tile([P, 8, D], BF16, tag="wq")
            wk_sb = wsb_pool.tile([P, 8, D], BF16, tag="wk")
            wv_sb = wsb_pool.tile([P, 2, 8, 512], BF16, tag="wv")
            xq_sb = xsb_pool.tile([P, 2, 8, 512], BF16, tag="xq")
            xk_sb = xsb_pool.tile([P, 2, 8, 512], BF16, tag="xk")
            vh = vh_pool.tile([P, 8, D], BF16, tag="vh")
            kth = vh_pool.tile([P, 8, 1024], BF16, tag="kth")

            # input loads, issued in consumption order (sync/HWDGE queue)
            nc.sync.dma_start(wv_sb[:, 0], wvd[0])
            nc.sync.dma_start(xq_sb[:, 0], xqd[0])
            nc.sync.dma_start(wk_sb[:], wkd[:])
            nc.sync.dma_start(xk_sb[:, 0], xkd[0])
            nc.sync.dma_start(xk_sb[:, 1], xkd[1])
            nc.sync.dma_start(wv_sb[:, 1], wvd[1])
            nc.sync.dma_start(xq_sb[:, 1], xqd[1])
            nc.sync.dma_start(wq_sb[:], wqd[:])
            nc.sync.dma_start(mask_sb[:], maskT[:])

            def emit_v(eh, hh):
                # V_own[k,e] for xq half hh, e-half eh
                for jj in range(4):
                    j = 4 * hh + jj
                    ps = pproj.tile([P, 512], F32, tag="pp")
                    for dt in range(8):
                        nc.tensor.matmul(
                            ps[:],
                            xq_sb[:, hh, dt, P * jj:P * (jj + 1)],
                            wv_sb[:, eh, dt, :],
                            start=(dt == 0), stop=(dt == 7))
                    nc.scalar.copy(vh[:, j, 512 * eh:512 * (eh + 1)], ps[:])

            emit_v(0, 0)

            # KT_own[e,k]: own half of the sequence (2 chunks of 512 keys)
            for c in range(2):
                for et in range(8):
                    ps = pproj.tile([P, 512], F32, tag="pp")
                    for dt in range(8):
                        nc.tensor.matmul(
                            ps[:], wk_sb[:, dt, P * et:P * (et + 1)],
                            xk_sb[:, c, dt, :],
                            start=(dt == 0), stop=(dt == 7))
                    nc.scalar.copy(kth[:, et, 512 * c:512 * (c + 1)], ps[:])

            # AllGather #1: KT halves
            cck_in = ccdram.tile([P, 8 * 1024], BF16, tag="cck_in")
            cck_out = ccdram.tile([2, P, 8 * 1024], BF16, tag="cck_out")
            nc.gpsimd.dma_start(cck_in[:],
                                kth[:].rearrange("p a b -> p (a b)"))
            nc.gpsimd.collective_compute(
                "AllGather", mybir.AluOpType.bypass,
                replica_groups=GROUPS,
                ins=[cck_in[:]], outs=[cck_out[:]])
            for r in range(2):
                nc.sync.dma_start(
                    KT4[:, r].rearrange("p a b -> p (a b)"), cck_out[r])

            # rest of V_own
            emit_v(1, 0)
            emit_v(0, 1)
            emit_v(1, 1)

            # AllGather #2: V halves
            ccv_in = ccdram.tile([P, 8 * D], BF16, tag="ccv_in")
            ccv_out = ccdram.tile([2, P, 8 * D], BF16, tag="ccv_out")
            nc.gpsimd.dma_start(ccv_in[:],
                                vh[:].rearrange("p a b -> p (a b)"))
            nc.gpsimd.collective_compute(
                "AllGather", mybir.AluOpType.bypass,
                replica_groups=GROUPS,
                ins=[ccv_in[:]], outs=[ccv_out[:]])
            for r in range(2):
                nc.sync.dma_start(
                    V4[:, r].rearrange("p a b -> p (a b)"), ccv_out[r])

            # QT[e,q]: stationary wq e-tile, moving xq half
            for et in range(8):
                for qh in range(2):
                    ps = pproj.tile([P, 512], F32, tag="pp")
                    for dt in range(8):
                        nc.tensor.matmul(
                            ps[:], wq_sb[:, dt, P * et:P * (et + 1)],
                            xq_sb[:, qh, dt, :],
                            start=(dt == 0), stop=(dt == 7))
                    nc.scalar.copy(QT[:, et, 512 * qh:512 * (qh + 1)], ps[:])

        # ---------------- attention phase ----------------
        # Slot pairs (2p, 2p+1) share k-tiles 0..nsh-1 at N=512; the later
        # slot's 4 extra k-tiles run at N=256 on the right half.  PV runs
        # with expT tiles stationary, producing out[q,e] directly (no PE
        # transposes); the softmax denominator is folded in via a DVE
        # tensor_scalar multiply during psum evacuation.
        with tc.tile_pool(name="ps_s", bufs=3, space="PSUM") as ps_s, \
             tc.tile_pool(name="ps_d", bufs=1, space="PSUM") as ps_d, \
             tc.tile_pool(name="ps_o", bufs=2, space="PSUM") as ps_o, \
             tc.tile_pool(name="expp", bufs=2) as expp, \
             tc.tile_pool(name="tmpp", bufs=2) as tmpp, \
             tc.tile_pool(name="osb", bufs=4) as osbp, \
             tc.tile_pool(name="dendram", bufs=1, space="DRAM") as dendramp, \
             tc.tile_pool(name="dsb", bufs=2) as dsbp:

            den_dram = dendramp.tile([2, 2 * QB], F32)

            def kslice(et, kt):
                # global 128-key tile kt -> gathered KT4 slice
                r, kk = divmod(kt, 8)
                return KT4[:, r, et, P * kk:P * (kk + 1)]

            for p in range(2):
                nsh = 4 * (2 * p + 1)      # shared k-tiles (slot 2p extent)
                ntot = nsh + 4             # + slot 2p+1's extra k-tiles
                expbuf = expp.tile([P, 16, 512], BF16, tag="expbuf")
                pd = ps_d.tile([P, 512], F32, tag="pd")

                # denominator matmul for tile k; emitted one iteration late
                # (software pipeline) so PE never waits on the exp latency
                def emit_den(k, pd=pd, nsh=nsh, ntot=ntot, expbuf=expbuf):
                    if k < nsh:
                        nc.tensor.matmul(pd[0:1, :], ones[:, 0:1],
                                         expbuf[:, k, :],
                                         start=(k == 0), stop=False)
                    else:
                        nc.tensor.matmul(pd[0:1, 256:512], ones[:, 0:1],
                                         expbuf[:, k, 0:256],
                                         start=False, stop=(k == ntot - 1))

                # scores + exp over the shared range at N=512
                for kt in range(nsh):
                    ps = ps_s.tile([P, 512], F32, tag="ps")
                    for et in range(8):
                        nc.tensor.matmul(
                            ps[:], kslice(et, kt),
                            QT[:, et, 512 * p:512 * (p + 1)],
                            start=(et == 0), stop=(et == 7))
                    j = kt - (nsh - 4)
                    if j >= 0:
                        tmp = tmpp.tile([P, 512], BF16, tag="tmp")
                        nc.scalar.activation(tmp[:], ps[:],
                                             mybir.ActivationFunctionType.Exp,
                                             scale=SCALE)
                        nc.vector.tensor_tensor(expbuf[:, kt, :], tmp[:],
                                                mask_sb[:, j, :],
                                                mybir.AluOpType.mult)
                    else:
                        nc.scalar.activation(expbuf[:, kt, :], ps[:],
                                             mybir.ActivationFunctionType.Exp,
                                             scale=SCALE)
                    if kt >= 1:
                        emit_den(kt - 1)

                # slot 2p+1's extra k-tiles at N=256 (right half)
                for ex in range(4):
                    kt = nsh + ex
                    ps = ps_s.tile([P, 512], F32, tag="ps")
                    for et in range(8):
                        nc.tensor.matmul(
                            ps[:, 0:256], kslice(et, kt),
                            QT[:, et, 512 * p + 256:512 * (p + 1)],
                            start=(et == 0), stop=(et == 7))
                    tmp = tmpp.tile([P, 512], BF16, tag="tmp")
                    nc.scalar.activation(tmp[:, 0:256], ps[:, 0:256],
                                         mybir.ActivationFunctionType.Exp,
                                         scale=SCALE)
                    nc.vector.tensor_tensor(expbuf[:, kt, 0:256],
                                            tmp[:, 0:256],
                                            mask_sb[:, ex, 0:256],
                                            mybir.AluOpType.mult)
                    emit_den(kt - 1)
                emit_den(ntot - 1)

                # denominator -> [q,1] layout via DRAM roundtrip
                dsb = dsbp.tile([1, 512], F32, tag="den")
                nc.vector.tensor_copy(dsb[:], pd[0:1, :])
                nc.sync.dma_start(den_dram[p:p + 1, :], dsb[:])
                nc.sync.dma_start(
                    denT[:, 4 * p:4 * p + 4],
                    den_dram[p:p + 1, :].rearrange("o (c q) -> (o q) c", q=P))
                nc.vector.reciprocal(rinv[:, 4 * p:4 * p + 4],
                                     denT[:, 4 * p:4 * p + 4])

                # PV per 128-query tile: expT stationary, V moving; psum
                # accumulates out[q, 1024e] across the tile's k range
                for qt in range(4):
                    kcnt = nsh if qt < 2 else ntot
                    po = ps_o.tile([P, 2, 512], F32, tag="po")
                    for kt in range(kcnt):
                        qo = P * qt if kt < nsh else P * (qt - 2)
                        t, rem = divmod(kt, 4)
                        r, jj = divmod(rem, 2)
                        jv = 2 * t + jj
                        for eh in range(2):
                            nc.tensor.matmul(
                                po[:, eh, :],
                                expbuf[:, kt, qo:qo + P],
                                V4[:, r, jv, 512 * eh:512 * (eh + 1)],
                                start=(kt == 0), stop=(kt == kcnt - 1))
                    ob = osbp.tile([P, D], BF16, tag="ob")
                    nc.vector.tensor_scalar_mul(
                        ob[:], po[:].rearrange("p a b -> p (a b)"),
                        rinv[:, 4 * p + qt:4 * p + qt + 1])
                    r0 = 512 * p + P * qt
                    nc.sync.dma_start(out[r0:r0 + P, :], ob[:])

    nc.compile()
    return nc


_NC_CACHE = None


def _get_nc():
    global _NC_CACHE
    if _NC_CACHE is None:
        _NC_CACHE = _build_kernel()
    return _NC_CACHE


def _make_masks():
    kk = np.arange(P)[:, None]
    qq = np.arange(256)[None, :]
    diag0 = (qq >= kk).astype(np.float32)
    diag1 = (qq >= kk + P).astype(np.float32)
    m = {}
    for h in range(2):
        mt = np.zeros((P, 4, 512), np.float32)
        mt[:, :, 256:] = 1.0  # right half (the later slot of a pair): allowed
        if h == 0:
            mt[:, 0, :256], mt[:, 1, :256] = diag0, diag1
        else:
            mt[:, 0, :256], mt[:, 1, :256] = 1.0, 1.0
            mt[:, 2, :256], mt[:, 3, :256] = diag0, diag1
        m[h] = mt.astype(ml_dtypes.bfloat16)
    return m


def _prep_inputs(x, Wq, Wk, Wv):
    bf16 = ml_dtypes.bfloat16

    def wfull(W):
        # [d_in, e] -> [p, dt, e]
        return np.ascontiguousarray(
            np.asarray(W, np.float32).reshape(8, P, D).transpose(1, 0, 2)
        ).astype(bf16)

    def whalves(W):
        # [d_in, e] -> [eh, p, dt, 512]
        wf = np.asarray(W, np.float32).reshape(8, P, 2, 512)
        return np.ascontiguousarray(wf.transpose(2, 1, 0, 3)).astype(bf16)

    wq4, wk4, wv4 = wfull(Wq), wfull(Wk), whalves(Wv)
    masks = _make_masks()
    in_maps = []
    for core in range(NCORES):
        b, h = divmod(core, 2)
        xb = np.asarray(x[b], np.float32)
        xt4 = np.ascontiguousarray(
            xb.reshape(4, 512, 8, P).transpose(0, 3, 2, 1)).astype(bf16)
        xk4 = np.ascontiguousarray(xt4[2 * h:2 * h + 2])
        order = np.concatenate(
            [np.arange(QB * (2 * t + h), QB * (2 * t + h) + QB)
             for t in range(NSLOT)])
        xq = xb[order]
        xq4 = np.ascontiguousarray(
            xq.reshape(2, 512, 8, P).transpose(0, 3, 2, 1)).astype(bf16)
        in_maps.append({
            "xkd": xk4, "xqd": xq4,
            "wqd": wq4, "wkd": wk4, "wvd": wv4,
            "maskT": masks[h],
        })
    return in_maps


def run(inputs, trace=False):
    nc = _get_nc()
    in_maps = _prep_inputs(inputs["x"], inputs["Wq"], inputs["Wk"],
                           inputs["Wv"])
    res = bass_utils.run_bass_kernel_spmd(
        nc, in_maps, core_ids=list(range(NCORES)), trace=trace)
    out = np.empty((B, S, D), np.float32)
    for core in range(NCORES):
        b, h = divmod(core, 2)
        oc = np.asarray(res.results[core]["out"]).astype(np.float32)
        for t in range(NSLOT):
            out[b, QB * (2 * t + h):QB * (2 * t + h) + QB] = \
                oc[QB * t:QB * t + QB]
    return out, res


def kernel(**inputs):
    out, _ = run(inputs, trace=False)
    return out


# revision 5
# speedup vs baseline: 1.1159x; 1.1159x over previous
"""Causal single-head attention (B=4, S=2048, D=1024) on 8 trn2 NeuronCores.

Sharding: core = (batch b, parity h).  Each core owns the 1024 queries of
batch b in 256-row blocks {2t+h : t=0..3} (interleaved for causal load
balance), projects Q for its own rows, K for the full sequence, and V for
its own rows only (V halves are pair-wise all-gathered, hidden under the
K projection).  K stays replicated: a gathered K would put the collective
on the scores critical path, and concurrent DMA traffic halves the
collective's effective bandwidth (measured), so the 27us of duplicated K
matmuls are cheaper.

All inputs are pre-cast to bf16 on the host (the on-chip matmuls are bf16
anyway), halving input HBM traffic and removing the f32 staging casts.

On-chip dataflow (per core, SPMD-uniform):
  warmup: 24 throwaway matmuls on a zeroed tile while the first input
          loads land, so the PE HAM clock-gate is at 8/8 when real work
          starts.
  proj:  V_own[k,e] = xq^T Wv   (8 row-tiles; gathered pair-wise via one
                                 AllGather into V4[r][j])
         QT[e,q]    = Wq^T xq^T
         KT[e,k]    = Wk^T x^T   (full sequence, 4 key chunks of 512)
  attention per slot pair p (queries 512p..512p+511, nsh = 4(2p+1)):
         scoresT[k,q] = KT^T QT  over k-tiles 0..nsh-1 at N=512
                        (+ 4 extra k-tiles at N=256 for the later slot)
         expT = exp(scoresT/32) * mask    (multiplicative 0/1 causal mask)
         den[1,q] += ones^T expT          (matmul; [q]-major via DRAM
                                           roundtrip -> rinv[q,1])
         out[q,e]  = sum_k expT[k,q]^T V[k,e]   <- PV with expT stationary,
                     accumulated per 128-query tile directly in [q,e]
                     orientation (no PE transposes), scaled by rinv on DVE
                     and stored bf16.
"""

import os
import sys
from contextlib import ExitStack

import numpy as np
import ml_dtypes

import concourse.bass as bass
import concourse.mybir as mybir
import concourse.tile as tile
from concourse import bacc
from concourse import bass_utils

B, S, D = 4, 2048, 1024
P = 128
QB = 256          # queries per slot
NSLOT = 4         # slots per core
NQ = QB * NSLOT   # queries per core
NCORES = 8
F32 = mybir.dt.float32
BF16 = mybir.dt.bfloat16
SCALE = 1.0 / 32.0  # 1/sqrt(D)


def _build_kernel():
    nc = bacc.Bacc("TRN2", target_bir_lowering=False, debug=False,
                   num_devices=NCORES)

    xtd = nc.dram_tensor("xtd", [4, P, 8, 512], BF16, kind="ExternalInput").ap()
    xqd = nc.dram_tensor("xqd", [2, P, 8, 512], BF16, kind="ExternalInput").ap()
    wqd = nc.dram_tensor("wqd", [P, 8, D], BF16, kind="ExternalInput").ap()
    wkd = nc.dram_tensor("wkd", [P, 8, D], BF16, kind="ExternalInput").ap()
    wvd = nc.dram_tensor("wvd", [2, P, 8, 512], BF16, kind="ExternalInput").ap()
    maskT = nc.dram_tensor("maskT", [P, 4, 512], BF16, kind="ExternalInput").ap()
    out = nc.dram_tensor("out", [NQ, D], BF16, kind="ExternalOutput").ap()
    # cores 2b (h=0) and 2b+1 (h=1) of batch b exchange V halves
    GROUPS = [[0, 1], [2, 3], [4, 5], [6, 7]]

    with tile.TileContext(nc) as tc, ExitStack() as ctx:
        const = ctx.enter_context(tc.tile_pool(name="const", bufs=1))
        persist = ctx.enter_context(tc.tile_pool(name="persist", bufs=1))

        ones = const.tile([P, 1], BF16)
        nc.gpsimd.memset(ones[:], 1.0)
        scratch = const.tile([P, 512], BF16)
        nc.gpsimd.memset(scratch[:], 0.0)
        mask_sb = const.tile([P, 4, 512], BF16)

        QT = persist.tile([P, 8, NQ], BF16)      # [e_in_tile, e_tile, q]
        KT = persist.tile([P, 8, S], BF16)       # [e_in_tile, e_tile, k]
        V4 = persist.tile([P, 2, 8, D], BF16)    # [k_in_tile, parity, j, e]
        denT = persist.tile([P, 2 * NSLOT], F32)
        rinv = persist.tile([P, 2 * NSLOT], F32)

        # ---------------- projection phase ----------------
        # V-own first so the pair AllGather launches as early as possible;
        # its latency hides under the Q and K projections.
        with tc.tile_pool(name="wsb", bufs=1) as wsb_pool, \
             tc.tile_pool(name="xtp", bufs=4) as xt_pool, \
             tc.tile_pool(name="xqp", bufs=1) as xq_pool, \
             tc.tile_pool(name="vhp", bufs=1) as vh_pool, \
             tc.tile_pool(name="ccdram", bufs=1, space="DRAM") as ccdram, \
             tc.tile_pool(name="pproj", bufs=4, space="PSUM") as pproj:

            wq_sb = wsb_pool.tile([P, 8, D], BF16, tag="wq")
            wk_sb = wsb_pool.tile([P, 8, D], BF16, tag="wk")
            wv_sb = wsb_pool.tile([P, 2, 8, 512], BF16, tag="wv")
            xq_sb = xq_pool.tile([P, 2, 8, 512], BF16, tag="xq")
            vh = vh_pool.tile([P, 8, D], BF16, tag="vh")

            # input loads, issued in consumption order (sync/HWDGE queue).
            # The first wv/xq halves are split by d-tile so the very first
            # matmul group can start after ~1MB instead of ~2MB.
            nc.sync.dma_start(wv_sb[:, 0, 0:4, :], wvd[0][:, 0:4, :])
            nc.sync.dma_start(xq_sb[:, 0, 0:4, :], xqd[0][:, 0:4, :])
            nc.sync.dma_start(wv_sb[:, 0, 4:8, :], wvd[0][:, 4:8, :])
            nc.sync.dma_start(xq_sb[:, 0, 4:8, :], xqd[0][:, 4:8, :])
            nc.sync.dma_start(xq_sb[:, 1], xqd[1])
            nc.sync.dma_start(wv_sb[:, 1], wvd[1])
            nc.sync.dma_start(wq_sb[:], wqd[:])
            nc.sync.dma_start(wk_sb[:], wkd[:])
            xt_tiles = []
            for c in range(4):
                xt = xt_pool.tile([P, 8, 512], BF16, tag="xt")
                nc.sync.dma_start(xt[:], xtd[c])
                xt_tiles.append(xt)
            nc.sync.dma_start(mask_sb[:], maskT[:])

            # PE warmup on zeroed data while the first loads are in flight
            wps = pproj.tile([P, 512], F32, tag="pp")
            for i in range(24):
                nc.tensor.matmul(wps[0:1, :], scratch[:, 0:1], scratch[:],
                                 start=(i == 0), stop=(i == 23))

            # V_own[k,e]: stationary xq row-tile, moving wv e-half.
            # The first (eh=0, hh=0) pass runs d-tile-half-major so its
            # 4 psum groups can start on the first half-loads.
            ps4 = [pproj.tile([P, 512], F32, tag="pp", name=f"ps4_{i}")
                   for i in range(4)]
            for dh in range(2):
                for jj in range(4):
                    for dt in range(4 * dh, 4 * dh + 4):
                        nc.tensor.matmul(
                            ps4[jj][:],
                            xq_sb[:, 0, dt, P * jj:P * (jj + 1)],
                            wv_sb[:, 0, dt, :],
                            start=(dt == 0), stop=(dt == 7))
            for jj in range(4):
                nc.scalar.copy(vh[:, jj, 0:512], ps4[jj][:])

            def emit_v(eh, hh):
                for jj in range(4):
                    j = 4 * hh + jj
                    ps = pproj.tile([P, 512], F32, tag="pp")
                    for dt in range(8):
                        nc.tensor.matmul(
                            ps[:],
                            xq_sb[:, hh, dt, P * jj:P * (jj + 1)],
                            wv_sb[:, eh, dt, :],
                            start=(dt == 0), stop=(dt == 7))
                    nc.scalar.copy(vh[:, j, 512 * eh:512 * (eh + 1)], ps[:])

            emit_v(0, 1)
            emit_v(1, 0)
            emit_v(1, 1)

            # pair all-gather of V halves (runs on TOPSP/SDMA, overlapped)
            cc_in = ccdram.tile([P, 8 * D], BF16, tag="cc_in")
            cc_out = ccdram.tile([2, P, 8 * D], BF16, tag="cc_out")
            nc.gpsimd.dma_start(cc_in[:],
                                vh[:].rearrange("p a b -> p (a b)"))
            nc.gpsimd.collective_compute(
                "AllGather", mybir.AluOpType.bypass,
                replica_groups=GROUPS,
                ins=[cc_in[:]], outs=[cc_out[:]])
            for r in range(2):
                nc.gpsimd.dma_start(
                    V4[:, r].rearrange("p a b -> p (a b)"), cc_out[r])

            # QT[e,q]: stationary wq e-tile, moving xq half
            for et in range(8):
                for qh in range(2):
                    ps = pproj.tile([P, 512], F32, tag="pp")
                    for dt in range(8):
                        nc.tensor.matmul(
                            ps[:], wq_sb[:, dt, P * et:P * (et + 1)],
                            xq_sb[:, qh, dt, :],
                            start=(dt == 0), stop=(dt == 7))
                    nc.scalar.copy(QT[:, et, 512 * qh:512 * (qh + 1)], ps[:])

            # KT[e,k]: full sequence, chunk by chunk as loads arrive
            for c in range(4):
                for et in range(8):
                    ps = pproj.tile([P, 512], F32, tag="pp")
                    for dt in range(8):
                        nc.tensor.matmul(
                            ps[:], wk_sb[:, dt, P * et:P * (et + 1)],
                            xt_tiles[c][:, dt, :],
                            start=(dt == 0), stop=(dt == 7))
                    nc.scalar.copy(KT[:, et, 512 * c:512 * (c + 1)], ps[:])

        # ---------------- attention phase ----------------
        # Slot pairs (2p, 2p+1) share k-tiles 0..nsh-1 at N=512; the later
        # slot's 4 extra k-tiles run at N=256 on the right half.  PV runs
        # with expT tiles stationary, producing out[q,e] directly (no PE
        # transposes); the softmax denominator is folded in via a DVE
        # tensor_scalar multiply during psum evacuation.
        with tc.tile_pool(name="ps_s", bufs=3, space="PSUM") as ps_s, \
             tc.tile_pool(name="ps_d", bufs=1, space="PSUM") as ps_d, \
             tc.tile_pool(name="ps_o", bufs=2, space="PSUM") as ps_o, \
             tc.tile_pool(name="expp", bufs=2) as expp, \
             tc.tile_pool(name="tmpp", bufs=2) as tmpp, \
             tc.tile_pool(name="osb", bufs=4) as osbp, \
             tc.tile_pool(name="dendram", bufs=1, space="DRAM") as dendramp, \
             tc.tile_pool(name="dsb", bufs=2) as dsbp:

            den_dram = dendramp.tile([2, 2 * QB], F32)

            for p in range(2):
                nsh = 4 * (2 * p + 1)      # shared k-tiles (slot 2p extent)
                ntot = nsh + 4             # + slot 2p+1's extra k-tiles
                expbuf = expp.tile([P, 16, 512], BF16, tag="expbuf")
                pd = ps_d.tile([P, 512], F32, tag="pd")

                # denominator matmul for tile k; emitted one iteration late
                # (software pipeline) so PE never waits on the exp latency
                def emit_den(k, pd=pd, nsh=nsh, ntot=ntot, expbuf=expbuf):
                    if k < nsh:
                        nc.tensor.matmul(pd[0:1, :], ones[:, 0:1],
                                         expbuf[:, k, :],
                                         start=(k == 0), stop=False)
                    else:
                        nc.tensor.matmul(pd[0:1, 256:512], ones[:, 0:1],
                                         expbuf[:, k, 0:256],
                                         start=False, stop=(k == ntot - 1))

                # scores + exp over the shared range at N=512
                for kt in range(nsh):
                    ps = ps_s.tile([P, 512], F32, tag="ps")
                    for et in range(8):
                        nc.tensor.matmul(
                            ps[:], KT[:, et, P * kt:P * (kt + 1)],
                            QT[:, et, 512 * p:512 * (p + 1)],
                            start=(et == 0), stop=(et == 7))
                    j = kt - (nsh - 4)
                    if j >= 0:
                        tmp = tmpp.tile([P, 512], BF16, tag="tmp")
                        nc.scalar.activation(tmp[:], ps[:],
                                             mybir.ActivationFunctionType.Exp,
                                             scale=SCALE)
                        nc.vector.tensor_tensor(expbuf[:, kt, :], tmp[:],
                                                mask_sb[:, j, :],
                                                mybir.AluOpType.mult)
                    else:
                        nc.scalar.activation(expbuf[:, kt, :], ps[:],
                                             mybir.ActivationFunctionType.Exp,
                                             scale=SCALE)
                    if kt >= 1:
                        emit_den(kt - 1)

                # slot 2p+1's extra k-tiles at N=256 (right half)
                for ex in range(4):
                    kt = nsh + ex
                    ps = ps_s.tile([P, 512], F32, tag="ps")
                    for et in range(8):
                        nc.tensor.matmul(
                            ps[:, 0:256], KT[:, et, P * kt:P * (kt + 1)],
                            QT[:, et, 512 * p + 256:512 * (p + 1)],
                            start=(et == 0), stop=(et == 7))
                    tmp = tmpp.tile([P, 512], BF16, tag="tmp")
                    nc.scalar.activation(tmp[:, 0:256], ps[:, 0:256],
                                         mybir.ActivationFunctionType.Exp,
                                         scale=SCALE)
                    nc.vector.tensor_tensor(expbuf[:, kt, 0:256],
                                            tmp[:, 0:256],
                                            mask_sb[:, ex, 0:256],
                                            mybir.AluOpType.mult)
                    emit_den(kt - 1)
                emit_den(ntot - 1)

                # denominator -> [q,1] layout via DRAM roundtrip
                dsb = dsbp.tile([1, 512], F32, tag="den")
                nc.vector.tensor_copy(dsb[:], pd[0:1, :])
                nc.sync.dma_start(den_dram[p:p + 1, :], dsb[:])
                nc.sync.dma_start(
                    denT[:, 4 * p:4 * p + 4],
                    den_dram[p:p + 1, :].rearrange("o (c q) -> (o q) c", q=P))
                nc.vector.reciprocal(rinv[:, 4 * p:4 * p + 4],
                                     denT[:, 4 * p:4 * p + 4])

                # PV per 128-query tile: expT stationary, V moving; psum
                # accumulates out[q, 1024e] across the tile's k range.
                # Evacuation is split per e-half so the store of the first
                # half overlaps the matmuls of the next tile.
                for qt in range(4):
                    kcnt = nsh if qt < 2 else ntot
                    po = ps_o.tile([P, 2, 512], F32, tag="po")
                    for kt in range(kcnt):
                        qo = P * qt if kt < nsh else P * (qt - 2)
                        t, rem = divmod(kt, 4)
                        r, jj = divmod(rem, 2)
                        jv = 2 * t + jj
                        for eh in range(2):
                            nc.tensor.matmul(
                                po[:, eh, :],
                                expbuf[:, kt, qo:qo + P],
                                V4[:, r, jv, 512 * eh:512 * (eh + 1)],
                                start=(kt == 0), stop=(kt == kcnt - 1))
                    ob = osbp.tile([P, D], BF16, tag="ob")
                    r0 = 512 * p + P * qt
                    for eh in range(2):
                        nc.vector.tensor_scalar_mul(
                            ob[:, 512 * eh:512 * (eh + 1)], po[:, eh, :],
                            rinv[:, 4 * p + qt:4 * p + qt + 1])
                        nc.sync.dma_start(
                            out[r0:r0 + P, 512 * eh:512 * (eh + 1)],
                            ob[:, 512 * eh:512 * (eh + 1)])

    nc.compile()
    return nc


_NC_CACHE = None


def _get_nc():
    global _NC_CACHE
    if _NC_CACHE is None:
        _NC_CACHE = _build_kernel()
    return _NC_CACHE


def _make_masks():
    kk = np.arange(P)[:, None]
    qq = np.arange(256)[None, :]
    diag0 = (qq >= kk).astype(np.float32)
    diag1 = (qq >= kk + P).astype(np.float32)
    m = {}
    for h in range(2):
        mt = np.zeros((P, 4, 512), np.float32)
        mt[:, :, 256:] = 1.0  # right half (the later slot of a pair): allowed
        if h == 0:
            mt[:, 0, :256], mt[:, 1, :256] = diag0, diag1
        else:
            mt[:, 0, :256], mt[:, 1, :256] = 1.0, 1.0
            mt[:, 2, :256], mt[:, 3, :256] = diag0, diag1
        m[h] = mt.astype(ml_dtypes.bfloat16)
    return m


def _prep_inputs(x, Wq, Wk, Wv):
    bf16 = ml_dtypes.bfloat16

    def wfull(W):
        # [d_in, e] -> [p, dt, e]
        return np.ascontiguousarray(
            np.asarray(W, np.float32).reshape(8, P, D).transpose(1, 0, 2)
        ).astype(bf16)

    def whalves(W):
        # [d_in, e] -> [eh, p, dt, 512]
        wf = np.asarray(W, np.float32).reshape(8, P, 2, 512)
        return np.ascontiguousarray(wf.transpose(2, 1, 0, 3)).astype(bf16)

    wq4, wk4, wv4 = wfull(Wq), wfull(Wk), whalves(Wv)
    masks = _make_masks()
    in_maps = []
    for core in range(NCORES):
        b, h = divmod(core, 2)
        xb = np.asarray(x[b], np.float32)
        xt4 = np.ascontiguousarray(
            xb.reshape(4, 512, 8, P).transpose(0, 3, 2, 1)).astype(bf16)
        order = np.concatenate(
            [np.arange(QB * (2 * t + h), QB * (2 * t + h) + QB)
             for t in range(NSLOT)])
        xq = xb[order]
        xq4 = np.ascontiguousarray(
            xq.reshape(2, 512, 8, P).transpose(0, 3, 2, 1)).astype(bf16)
        in_maps.append({
            "xtd": xt4, "xqd": xq4,
            "wqd": wq4, "wkd": wk4, "wvd": wv4,
            "maskT": masks[h],
        })
    return in_maps


def run(inputs, trace=False):
    nc = _get_nc()
    in_maps = _prep_inputs(inputs["x"], inputs["Wq"], inputs["Wk"],
                           inputs["Wv"])
    res = bass_utils.run_bass_kernel_spmd(
        nc, in_maps, core_ids=list(range(NCORES)), trace=trace)
    out = np.empty((B, S, D), np.float32)
    for core in range(NCORES):
        b, h = divmod(core, 2)
        oc = np.asarray(res.results[core]["out"]).astype(np.float32)
        for t in range(NSLOT):
            out[b, QB * (2 * t + h):QB * (2 * t + h) + QB] = \
                oc[QB * t:QB * t + QB]
    return out, res


def kernel(**inputs):
    out, _ = run(inputs, trace=False)
    return out


# revision 12
# speedup vs baseline: 1.1302x; 1.0128x over previous
"""Causal single-head attention (B=4, S=2048, D=1024) on 8 trn2 NeuronCores.

Sharding: core = (batch b, parity h).  Each core owns the 1024 queries of
batch b in 256-row blocks {2t+h : t=0..3} (interleaved for causal load
balance), projects Q for its own rows, K for the full sequence, and V for
its own rows only (V halves are pair-wise all-gathered, hidden under the
K projection).  K stays replicated: a gathered K would put the collective
on the scores critical path, and concurrent DMA traffic halves the
collective's effective bandwidth (measured), so the 27us of duplicated K
matmuls are cheaper.

All inputs are pre-cast to bf16 on the host (the on-chip matmuls are bf16
anyway), halving input HBM traffic and removing the f32 staging casts.

On-chip dataflow (per core, SPMD-uniform):
  warmup: 24 throwaway matmuls on a zeroed tile while the first input
          loads land, so the PE HAM clock-gate is at 8/8 when real work
          starts.
  proj:  V_own[k,e] = xq^T Wv   (8 row-tiles; gathered pair-wise via one
                                 AllGather into V4[r][j])
         QT[e,q]    = Wq^T xq^T
         KT[e,k]    = Wk^T x^T   (full sequence, 4 key chunks of 512)
  attention per slot pair p (queries 512p..512p+511, nsh = 4(2p+1)):
         scoresT[k,q] = KT^T QT  over k-tiles 0..nsh-1 at N=512
                        (+ 4 extra k-tiles at N=256 for the later slot)
         expT = exp(scoresT/32) * mask    (multiplicative 0/1 causal mask)
         den[1,q] += ones^T expT          (matmul; [q]-major via DRAM
                                           roundtrip -> rinv[q,1])
         out[q,e]  = sum_k expT[k,q]^T V[k,e]   <- PV with expT stationary,
                     accumulated per 128-query tile directly in [q,e]
                     orientation (no PE transposes), scaled by rinv on DVE
                     and stored bf16.
"""

import os
import sys
from contextlib import ExitStack

import numpy as np
import ml_dtypes

import concourse.bass as bass
import concourse.mybir as mybir
import concourse.tile as tile
from concourse import bacc
from concourse import bass_utils

B, S, D = 4, 2048, 1024
P = 128
QB = 256          # queries per slot
NSLOT = 4         # slots per core
NQ = QB * NSLOT   # queries per core
NCORES = 8
F32 = mybir.dt.float32
BF16 = mybir.dt.bfloat16
SCALE = 1.0 / 32.0  # 1/sqrt(D)


def _build_kernel():
    nc = bacc.Bacc("TRN2", target_bir_lowering=False, debug=False,
                   num_devices=NCORES)

    xtd = nc.dram_tensor("xtd", [4, P, 8, 512], BF16, kind="ExternalInput").ap()
    xqd = nc.dram_tensor("xqd", [2, P, 8, 512], BF16, kind="ExternalInput").ap()
    wqd = nc.dram_tensor("wqd", [P, 8, D], BF16, kind="ExternalInput").ap()
    wkd = nc.dram_tensor("wkd", [P, 8, D], BF16, kind="ExternalInput").ap()
    wvd = nc.dram_tensor("wvd", [2, P, 8, 512], BF16, kind="ExternalInput").ap()
    maskT = nc.dram_tensor("maskT", [P, 4, 512], BF16, kind="ExternalInput").ap()
    out = nc.dram_tensor("out", [NQ, D], BF16, kind="ExternalOutput").ap()
    # cores 2b (h=0) and 2b+1 (h=1) of batch b exchange V halves
    GROUPS = [[0, 1], [2, 3], [4, 5], [6, 7]]

    with tile.TileContext(nc) as tc, ExitStack() as ctx:
        const = ctx.enter_context(tc.tile_pool(name="const", bufs=1))
        persist = ctx.enter_context(tc.tile_pool(name="persist", bufs=1))

        ones = const.tile([P, 1], BF16)
        nc.gpsimd.memset(ones[:], 1.0)
        scratch = const.tile([P, 512], BF16)
        nc.gpsimd.memset(scratch[:], 0.0)
        mask_sb = const.tile([P, 4, 512], BF16)

        QT = persist.tile([P, 8, NQ], BF16)      # [e_in_tile, e_tile, q]
        KT = persist.tile([P, 8, S], BF16)       # [e_in_tile, e_tile, k]
        V4 = persist.tile([P, 2, 8, D], BF16)    # [k_in_tile, parity, j, e]
        denT = persist.tile([P, 2 * NSLOT], F32)
        rinv = persist.tile([P, 2 * NSLOT], F32)

        # ---------------- projection phase ----------------
        # V-own first so the pair AllGather launches as early as possible;
        # its latency hides under the Q and K projections.
        with tc.tile_pool(name="wsb", bufs=1) as wsb_pool, \
             tc.tile_pool(name="xtp", bufs=4) as xt_pool, \
             tc.tile_pool(name="xqp", bufs=1) as xq_pool, \
             tc.tile_pool(name="vhp", bufs=1) as vh_pool, \
             tc.tile_pool(name="ccdram", bufs=1, space="DRAM") as ccdram, \
             tc.tile_pool(name="pproj", bufs=4, space="PSUM") as pproj:

            wq_sb = wsb_pool.tile([P, 8, D], BF16, tag="wq")
            wk_sb = wsb_pool.tile([P, 8, D], BF16, tag="wk")
            wv_sb = wsb_pool.tile([P, 2, 8, 512], BF16, tag="wv")
            xq_sb = xq_pool.tile([P, 2, 8, 512], BF16, tag="xq")
            vh = vh_pool.tile([P, 8, D], BF16, tag="vh")

            # input loads, issued in consumption order (sync/HWDGE queue).
            # The first wv/xq halves are split by d-tile so the very first
            # matmul group can start after ~1MB instead of ~2MB.
            nc.sync.dma_start(wv_sb[:, 0, 0:4, :], wvd[0][:, 0:4, :])
            nc.sync.dma_start(xq_sb[:, 0, 0:4, :], xqd[0][:, 0:4, :])
            nc.sync.dma_start(wv_sb[:, 0, 4:8, :], wvd[0][:, 4:8, :])
            nc.sync.dma_start(xq_sb[:, 0, 4:8, :], xqd[0][:, 4:8, :])
            nc.sync.dma_start(xq_sb[:, 1], xqd[1])
            nc.sync.dma_start(wv_sb[:, 1], wvd[1])
            nc.sync.dma_start(wq_sb[:], wqd[:])
            nc.sync.dma_start(wk_sb[:], wkd[:])
            xt_tiles = []
            for c in range(4):
                xt = xt_pool.tile([P, 8, 512], BF16, tag="xt")
                nc.sync.dma_start(xt[:], xtd[c])
                xt_tiles.append(xt)
            nc.sync.dma_start(mask_sb[:], maskT[:])

            # PE warmup on zeroed data while the first loads are in flight
            wps = pproj.tile([P, 512], F32, tag="pp")
            for i in range(16):
                nc.tensor.matmul(wps[0:1, :], scratch[:, 0:1], scratch[:],
                                 start=(i == 0), stop=(i == 15))

            # V_own[k,e]: stationary xq row-tile, moving wv e-half.
            # The first (eh=0, hh=0) pass runs d-tile-half-major so its
            # 4 psum groups can start on the first half-loads.
            ps4 = [pproj.tile([P, 512], F32, tag="pp", name=f"ps4_{i}")
                   for i in range(4)]
            for dh in range(2):
                for jj in range(4):
                    for dt in range(4 * dh, 4 * dh + 4):
                        nc.tensor.matmul(
                            ps4[jj][:],
                            xq_sb[:, 0, dt, P * jj:P * (jj + 1)],
                            wv_sb[:, 0, dt, :],
                            start=(dt == 0), stop=(dt == 7))
            for jj in range(4):
                nc.scalar.copy(vh[:, jj, 0:512], ps4[jj][:])

            def emit_v(eh, hh):
                for jj in range(4):
                    j = 4 * hh + jj
                    ps = pproj.tile([P, 512], F32, tag="pp")
                    for dt in range(8):
                        nc.tensor.matmul(
                            ps[:],
                            xq_sb[:, hh, dt, P * jj:P * (jj + 1)],
                            wv_sb[:, eh, dt, :],
                            start=(dt == 0), stop=(dt == 7))
                    nc.scalar.copy(vh[:, j, 512 * eh:512 * (eh + 1)], ps[:])

            emit_v(0, 1)
            emit_v(1, 0)
            emit_v(1, 1)

            # pair all-gather of V halves (runs on TOPSP/SDMA, overlapped)
            cc_in = ccdram.tile([P, 8 * D], BF16, tag="cc_in")
            cc_out = ccdram.tile([2, P, 8 * D], BF16, tag="cc_out")
            nc.gpsimd.dma_start(cc_in[:],
                                vh[:].rearrange("p a b -> p (a b)"))
            nc.gpsimd.collective_compute(
                "AllGather", mybir.AluOpType.bypass,
                replica_groups=GROUPS,
                ins=[cc_in[:]], outs=[cc_out[:]])
            for r in range(2):
                nc.gpsimd.dma_start(
                    V4[:, r].rearrange("p a b -> p (a b)"), cc_out[r])

            # QT[e,q]: stationary wq e-tile, moving xq half
            for et in range(8):
                for qh in range(2):
                    ps = pproj.tile([P, 512], F32, tag="pp")
                    for dt in range(8):
                        nc.tensor.matmul(
                            ps[:], wq_sb[:, dt, P * et:P * (et + 1)],
                            xq_sb[:, qh, dt, :],
                            start=(dt == 0), stop=(dt == 7))
                    nc.scalar.copy(QT[:, et, 512 * qh:512 * (qh + 1)], ps[:])

            # KT[e,k]: full sequence, chunk by chunk as loads arrive
            for c in range(4):
                for et in range(8):
                    ps = pproj.tile([P, 512], F32, tag="pp")
                    for dt in range(8):
                        nc.tensor.matmul(
                            ps[:], wk_sb[:, dt, P * et:P * (et + 1)],
                            xt_tiles[c][:, dt, :],
                            start=(dt == 0), stop=(dt == 7))
                    nc.scalar.copy(KT[:, et, 512 * c:512 * (c + 1)], ps[:])

        # ---------------- attention phase ----------------
        # Slot pairs (2p, 2p+1) share k-tiles 0..nsh-1 at N=512; the later
        # slot's 4 extra k-tiles run at N=256 on the right half.  PV runs
        # with expT tiles stationary, producing out[q,e] directly (no PE
        # transposes); the softmax denominator is folded in via a DVE
        # tensor_scalar multiply during psum evacuation.
        with tc.tile_pool(name="ps_s", bufs=2, space="PSUM") as ps_s, \
             tc.tile_pool(name="ps_d", bufs=1, space="PSUM") as ps_d, \
             tc.tile_pool(name="ps_o", bufs=2, space="PSUM") as ps_o, \
             tc.tile_pool(name="expp", bufs=2) as expp, \
             tc.tile_pool(name="tmpp", bufs=2) as tmpp, \
             tc.tile_pool(name="osb", bufs=4) as osbp, \
             tc.tile_pool(name="dendram", bufs=1, space="DRAM") as dendramp, \
             tc.tile_pool(name="dsb", bufs=2) as dsbp:

            den_dram = dendramp.tile([2, 2 * QB], F32)

            for p in range(2):
                nsh = 4 * (2 * p + 1)      # shared k-tiles (slot 2p extent)
                ntot = nsh + 4             # + slot 2p+1's extra k-tiles
                expbuf = expp.tile([P, 16, 512], BF16, tag="expbuf")
                pd_l = ps_d.tile([P, 512], F32, tag="pdl")
                pd_r = ps_d.tile([P, 512], F32, tag="pdr")

                # denominator matmuls for tile k; emitted one iteration late
                # (software pipeline) so PE never waits on the exp latency.
                # Left (earlier) slot and right slot accumulate in separate
                # psum banks so the left half's reciprocal is ready (and
                # readable without a bank hazard) as soon as the shared
                # range ends, unblocking the first PV evacuations.
                def emit_den(k, pd_l=pd_l, pd_r=pd_r, nsh=nsh, ntot=ntot,
                             expbuf=expbuf):
                    if k < nsh:
                        nc.tensor.matmul(pd_l[0:1, 0:256], ones[:, 0:1],
                                         expbuf[:, k, 0:256],
                                         start=(k == 0), stop=(k == nsh - 1))
                        nc.tensor.matmul(pd_r[0:1, 0:256], ones[:, 0:1],
                                         expbuf[:, k, 256:512],
                                         start=(k == 0), stop=False)
                    else:
                        nc.tensor.matmul(pd_r[0:1, 0:256], ones[:, 0:1],
                                         expbuf[:, k, 0:256],
                                         start=False, stop=(k == ntot - 1))

                # scores + exp over the shared range at N=512
                for kt in range(nsh):
                    ps = ps_s.tile([P, 512], F32, tag="ps")
                    for et in range(8):
                        nc.tensor.matmul(
                            ps[:], KT[:, et, P * kt:P * (kt + 1)],
                            QT[:, et, 512 * p:512 * (p + 1)],
                            start=(et == 0), stop=(et == 7))
                    j = kt - (nsh - 4)
                    if j >= 0:
                        tmp = tmpp.tile([P, 512], BF16, tag="tmp")
                        nc.scalar.activation(tmp[:], ps[:],
                                             mybir.ActivationFunctionType.Exp,
                                             scale=SCALE)
                        nc.vector.tensor_tensor(expbuf[:, kt, :], tmp[:],
                                                mask_sb[:, j, :],
                                                mybir.AluOpType.mult)
                    else:
                        nc.scalar.activation(expbuf[:, kt, :], ps[:],
                                             mybir.ActivationFunctionType.Exp,
                                             scale=SCALE)
                    if kt >= 1:
                        emit_den(kt - 1)

                # denominator -> [q,1] layout via DRAM roundtrip, one half
                # (slot) at a time: the left half's roundtrip is emitted as
                # soon as its accumulation stops, so its rinv is ready when
                # the first PV evacuation needs it
                dsb = dsbp.tile([1, 2, 256], F32, tag="den", name=f"dsb{p}")

                def emit_denrt(sh, p=p, dsb=dsb, pd_l=pd_l, pd_r=pd_r):
                    nc.vector.tensor_copy(dsb[:, sh],
                                          (pd_l if sh == 0 else
                                           pd_r)[0:1, 0:256])
                    nc.sync.dma_start(den_dram[p:p + 1, 256 * sh:
                                               256 * (sh + 1)],
                                      dsb[:, sh])
                    nc.sync.dma_start(
                        denT[:, 4 * p + 2 * sh:4 * p + 2 * sh + 2],
                        den_dram[p:p + 1, 256 * sh:256 * (sh + 1)]
                        .rearrange("o (c q) -> (o q) c", q=P))
                    nc.vector.reciprocal(
                        rinv[:, 4 * p + 2 * sh:4 * p + 2 * sh + 2],
                        denT[:, 4 * p + 2 * sh:4 * p + 2 * sh + 2])

                # slot 2p+1's extra k-tiles at N=256 (right half)
                for ex in range(4):
                    kt = nsh + ex
                    ps = ps_s.tile([P, 512], F32, tag="ps")
                    for et in range(8):
                        nc.tensor.matmul(
                            ps[:, 0:256], KT[:, et, P * kt:P * (kt + 1)],
                            QT[:, et, 512 * p + 256:512 * (p + 1)],
                            start=(et == 0), stop=(et == 7))
                    tmp = tmpp.tile([P, 512], BF16, tag="tmp")
                    nc.scalar.activation(tmp[:, 0:256], ps[:, 0:256],
                                         mybir.ActivationFunctionType.Exp,
                                         scale=SCALE)
                    nc.vector.tensor_tensor(expbuf[:, kt, 0:256],
                                            tmp[:, 0:256],
                                            mask_sb[:, ex, 0:256],
                                            mybir.AluOpType.mult)
                    emit_den(kt - 1)
                    if ex == 0:
                        emit_denrt(0)
                emit_den(ntot - 1)
                emit_denrt(1)

                # PV per 128-query tile: expT stationary, V moving; psum
                # accumulates out[q, 1024e] across the tile's k range.
                # Evacuation is split per e-half so the store of the first
                # half overlaps the matmuls of the next tile.
                for qt in range(4):
                    kcnt = nsh if qt < 2 else ntot
                    po = ps_o.tile([P, 2, 512], F32, tag="po")
                    for kt in range(kcnt):
                        qo = P * qt if kt < nsh else P * (qt - 2)
                        t, rem = divmod(kt, 4)
                        r, jj = divmod(rem, 2)
                        jv = 2 * t + jj
                        for eh in range(2):
                            nc.tensor.matmul(
                                po[:, eh, :],
                                expbuf[:, kt, qo:qo + P],
                                V4[:, r, jv, 512 * eh:512 * (eh + 1)],
                                start=(kt == 0), stop=(kt == kcnt - 1))
                    ob = osbp.tile([P, D], BF16, tag="ob")
                    r0 = 512 * p + P * qt
                    for eh in range(2):
                        nc.vector.tensor_scalar_mul(
                            ob[:, 512 * eh:512 * (eh + 1)], po[:, eh, :],
                            rinv[:, 4 * p + qt:4 * p + qt + 1])
                        nc.sync.dma_start(
                            out[r0:r0 + P, 512 * eh:512 * (eh + 1)],
                            ob[:, 512 * eh:512 * (eh + 1)])

    nc.compile()
    return nc


_NC_CACHE = None


def _get_nc():
    global _NC_CACHE
    if _NC_CACHE is None:
        _NC_CACHE = _build_kernel()
    return _NC_CACHE


def _make_masks():
    kk = np.arange(P)[:, None]
    qq = np.arange(256)[None, :]
    diag0 = (qq >= kk).astype(np.float32)
    diag1 = (qq >= kk + P).astype(np.float32)
    m = {}
    for h in range(2):
        mt = np.zeros((P, 4, 512), np.float32)
        mt[:, :, 256:] = 1.0  # right half (the later slot of a pair): allowed
        if h == 0:
            mt[:, 0, :256], mt[:, 1, :256] = diag0, diag1
        else:
            mt[:, 0, :256], mt[:, 1, :256] = 1.0, 1.0
            mt[:, 2, :256], mt[:, 3, :256] = diag0, diag1
        m[h] = mt.astype(ml_dtypes.bfloat16)
    return m


def _prep_inputs(x, Wq, Wk, Wv):
    bf16 = ml_dtypes.bfloat16

    def wfull(W):
        # [d_in, e] -> [p, dt, e]
        return np.ascontiguousarray(
            np.asarray(W, np.float32).reshape(8, P, D).transpose(1, 0, 2)
        ).astype(bf16)

    def whalves(W):
        # [d_in, e] -> [eh, p, dt, 512]
        wf = np.asarray(W, np.float32).reshape(8, P, 2, 512)
        return np.ascontiguousarray(wf.transpose(2, 1, 0, 3)).astype(bf16)

    wq4, wk4, wv4 = wfull(Wq), wfull(Wk), whalves(Wv)
    masks = _make_masks()
    in_maps = []
    for core in range(NCORES):
        b, h = divmod(core, 2)
        xb = np.asarray(x[b], np.float32)
        xt4 = np.ascontiguousarray(
            xb.reshape(4, 512, 8, P).transpose(0, 3, 2, 1)).astype(bf16)
        order = np.concatenate(
            [np.arange(QB * (2 * t + h), QB * (2 * t + h) + QB)
             for t in range(NSLOT)])
        xq = xb[order]
        xq4 = np.ascontiguousarray(
            xq.reshape(2, 512, 8, P).transpose(0, 3, 2, 1)).astype(bf16)
        in_maps.append({
            "xtd": xt4, "xqd": xq4,
            "wqd": wq4, "wkd": wk4, "wvd": wv4,
            "maskT": masks[h],
        })
    return in_maps


def run(inputs, trace=False):
    nc = _get_nc()
    in_maps = _prep_inputs(inputs["x"], inputs["Wq"], inputs["Wk"],
                           inputs["Wv"])
    res = bass_utils.run_bass_kernel_spmd(
        nc, in_maps, core_ids=list(range(NCORES)), trace=trace)
    out = np.empty((B, S, D), np.float32)
    for core in range(NCORES):
        b, h = divmod(core, 2)
        oc = np.asarray(res.results[core]["out"]).astype(np.float32)
        for t in range(NSLOT):
            out[b, QB * (2 * t + h):QB * (2 * t + h) + QB] = \
                oc[QB * t:QB * t + QB]
    return out, res


def kernel(**inputs):
    out, _ = run(inputs, trace=False)
    return out
